# revision 1
# baseline (speedup 1.0000x reference)
"""Multi-head attention (B=4, N=1024, C=1024, H=16, D=64) on 8 Trainium2 cores.

Sharding: query-parallel, no collectives. Core i handles batch b = i//2 and
query rows (i%2)*512..+512 (the host rolls x[b] so each core's query rows come
first; key order is irrelevant to softmax). Each core computes k/v for its
whole batch, attention for its 512 query rows over all 16 heads, and the
output projection for those rows. The host concatenates the 8 row-slices --
softmax rows are independent, so no cross-core reduction is needed.

Matmuls run in fp16 (1 PE cycle/row vs 1.5 for fp32r, FWL weight loads,
half-size DMA, 10-bit mantissa). Accumulation is always fp32 in PSUM. exp is
computed as exp(s/8 - 12*ln2) so unnormalized attention outputs stay in fp16
range; the 2^-12 factor cancels in the softmax normalization.

Per-core pipeline (x^T is prepared on the host -- input marshalling):
  1. v = x @ w_v -> [m, h, d] with an appended ones column, key mask folded in.
  2. Per head pair t: k^T/q^T column projections for pair t only, then
     S^T = k^T.T @ q^T (K=64 row-tiles, the pair alternates PE row groups
     0/64 so matmuls overlap) -> exp on ACT -> out^T (unnormalized) and the
     softmax denominator via the ones column -> stage to SBUF.
     Attention of pair t overlaps projections of pair t+1 on the PE.
  3. Per head quad: one DVE reciprocal of the 4 denominators, gpsimd
     partition-broadcast, DVE in-place normalize of out^T.
  4. y = o^T.T @ w_out + bias (K=1 ones-row matmul), DMA out.
"""

import os

import numpy as np

import concourse.bacc as bacc
import concourse.mybir as mybir
import concourse.tile as tile
from concourse.bass_utils import run_bass_kernel_spmd

F32 = mybir.dt.float32
F16 = mybir.dt.float16

B, N, C = 4, 1024, 1024
H, D = 16, 64
NQ = 512          # query rows per core
P = 128
MO = N // P       # 8 key m-tiles
CO = C // P       # 8 contraction tiles
EO = C // P       # 8 e-tiles for q (and separately k)
NO = NQ // P      # 4 output row tiles
ATT_SCALE = D ** -0.5
EXP_BIAS = float(-12.0 * np.log(2.0))  # keep out^T in fp16 range
N_CORES = 8


def build_nc():
    nc = bacc.Bacc()
    xbT = nc.declare_dram_parameter("xbT", [C, N], F16, isOutput=False)
    maskb = nc.declare_dram_parameter("maskb", [N], F32, isOutput=False)
    wq_pk = nc.declare_dram_parameter("wq_pk", [H // 2, P, CO, P], F16,
                                      isOutput=False)
    wk_pk = nc.declare_dram_parameter("wk_pk", [H // 2, P, CO, P], F16,
                                      isOutput=False)
    wv_pk = nc.declare_dram_parameter("wv_pk", [2, P, CO, NQ], F16,
                                      isOutput=False)
    wo_pk = nc.declare_dram_parameter("wo_pk", [P, EO, C], F16, isOutput=False)
    b_out = nc.declare_dram_parameter("b_out", [C], F16, isOutput=False)
    y = nc.declare_dram_parameter("y", [NQ, C], F32, isOutput=True)

    xbT_t = xbT.rearrange("(co p) m -> p co m", p=P)
    y_t = y.rearrange("(no p) c -> p no c", p=P)

    with tile.TileContext(nc) as tc:
        with tc.tile_pool(name="consts", bufs=1) as consts, \
             tc.tile_pool(name="persist", bufs=1) as persist:
            # ---- constants ----
            ones_row = consts.tile([1, P], F16)      # K=1 bias matmul lhsT
            nc.vector.memset(ones_row[:], 1.0)
            onesH = consts.tile([P, H], F16)
            nc.vector.memset(onesH[:], 1.0)
            ones_q = consts.tile([97, 64], F16)
            nc.vector.memset(ones_q[:], 1.0)
            mask_sb = consts.tile([P, MO], F32)
            nc.sync.dma_start(mask_sb[:], maskb.rearrange("(o p) -> p o", p=P))
            ebias = consts.tile([P, 1], F32)
            nc.vector.memset(ebias[:], EXP_BIAS)
            bias_sb = consts.tile([1, C], F16)
            nc.sync.dma_start(bias_sb[:], b_out[None, :])

            # ---- persistent tensors ----
            qT = persist.tile([P, EO, NQ], F16)          # q^T: [e, n]
            kT = persist.tile([P, EO, N], F16)           # k^T: [e, m]
            v_sb = persist.tile([P, MO, H, D + 1], F16)  # v + ones col
            o_sb = persist.tile([P, EO, NQ], F16)        # out^T: [e, n]
            wo = persist.tile([P, EO, C], F16)           # w_out staged early

            with tc.tile_pool(name="xT_pool", bufs=1) as xT_pool, \
                 tc.tile_pool(name="w_pool", bufs=4) as w_pool, \
                 tc.tile_pool(name="E_pool", bufs=3) as E_pool, \
                 tc.tile_pool(name="sm_pool", bufs=2) as sm_pool, \
                 tc.tile_pool(name="ou_pool", bufs=6) as ou_pool, \
                 tc.tile_pool(name="ps_proj", bufs=2, space="PSUM") as ps_proj, \
                 tc.tile_pool(name="ps_s", bufs=1, space="PSUM") as ps_s, \
                 tc.tile_pool(name="ps_av", bufs=2, space="PSUM") as ps_av:
                xT = xT_pool.tile([P, CO, N], F16)       # x^T: [c, m]
                nc.sync.dma_start(xT[:, 0, :], xbT_t[:, 0, :])

                def kq_proj(t, wk=None, wq=None):
                    # k^T columns for pair t (k block starts at e = C)
                    if wk is None:
                        wk = w_pool.tile([P, CO, P], F16, tag="wqk",
                                         name=f"wk{t}")
                        nc.sync.dma_start(wk[:], wk_pk[t])
                    for half in range(2):
                        pk = ps_proj.tile([P, NQ], F32, tag="pp", name=f"pk{t}_{half}")
                        for co in range(CO):
                            nc.tensor.matmul(
                                pk[:], wk[:, co, :],
                                xT[:, co, half * NQ:(half + 1) * NQ],
                                start=(co == 0), stop=(co == CO - 1))
                        nc.vector.tensor_copy(
                            kT[:, t, half * NQ:(half + 1) * NQ], pk[:])
                    if wq is None:
                        wq = w_pool.tile([P, CO, P], F16, tag="wqk",
                                         name=f"wq{t}")
                        nc.sync.dma_start(wq[:], wq_pk[t])
                    pq = ps_proj.tile([P, NQ], F32, tag="pp", name=f"pq{t}")
                    for co in range(CO):
                        nc.tensor.matmul(
                            pq[:], wq[:, co, :], xT[:, co, 0:NQ],
                            start=(co == 0), stop=(co == CO - 1))
                    nc.vector.tensor_copy(qT[:, t, :], pq[:])

                def s_exp(t):
                    # S^T + exp: one [128,1024] psum per mo holds both heads of
                    # the pair (row groups 0/64 -> the two matmuls overlap)
                    E_pair = E_pool.tile([P, MO, 2, NQ], F16, tag="E",
                                         name=f"E{t}")
                    for mp in range(MO // 2):
                        pss = ps_s.tile([P, 4 * NQ], F32, tag="ps_s",
                                        name=f"pss{t}_{mp}")
                        for sub in range(2):
                            mo = 2 * mp + sub
                            for j in range(2):
                                pb = 64 * j
                                nc.tensor.matmul(
                                    pss[:, (2 * sub + j) * NQ:
                                        (2 * sub + j + 1) * NQ],
                                    kT[pb:pb + 64, t, mo * P:(mo + 1) * P],
                                    qT[pb:pb + 64, t, :],
                                    start=True, stop=True)
                        nc.scalar.activation(
                            E_pair[:, 2 * mp:2 * mp + 2, :, :], pss[:],
                            mybir.ActivationFunctionType.Exp,
                            bias=ebias[:], scale=ATT_SCALE)
                    return E_pair

                norm_state = {}

                def av_norm(t, E_pair):
                    # AV + stage unnormalized out^T and denominator
                    if t % 2 == 0:
                        den_q = sm_pool.tile([97, NQ], F32, tag="den",
                                             name=f"den{t}")
                        nc.vector.memset(den_q[:], 1.0)
                        norm_state["den"] = den_q
                        norm_state["ou"] = []
                    den_q = norm_state["den"]
                    for j in range(2):
                        h = 2 * t + j
                        pb = 64 * j
                        pav = ps_av.tile([P, NQ], F32, tag="ps_av",
                                         name=f"pav{h}")
                        for mo in range(MO):
                            nc.tensor.matmul(
                                pav[0:D + 1, :], v_sb[:, mo, h, :],
                                E_pair[:, mo, j, :],
                                start=(mo == 0), stop=(mo == MO - 1))
                        o_un = ou_pool.tile([P, NQ], F16, tag="ou",
                                            name=f"ou{h}")
                        norm_state["ou"].append(o_un)
                        nc.vector.tensor_copy(o_un[pb:pb + 64, :], pav[0:D, :])
                        nc.vector.tensor_copy(
                            den_q[32 * (h % 4):32 * (h % 4) + 1, :],
                            pav[D:D + 1, :])
                    # normalize the completed quad
                    if t % 2 == 1:
                        rcp_q = sm_pool.tile([97, NQ], F16, tag="rcp",
                                             name=f"rcp{t}")
                        with nc.allow_low_precision(
                                reason="softmax 1/den in fp16 (~5e-4)"):
                            nc.vector.reciprocal(rcp_q[:], den_q[:])
                        for r in range(4):
                            h = 4 * (t // 2) + r
                            tt, pb = h // 2, 64 * (h % 2)
                            pbc = ps_av.tile([64, NQ], F32, tag="ps_av",
                                             name=f"pbc{h}")
                            nc.tensor.matmul(
                                pbc[:], ones_q[32 * r:32 * r + 1, :],
                                rcp_q[32 * r:32 * r + 1, :],
                                start=True, stop=True,
                                tile_position=(32 * r, 0))
                            nc.vector.tensor_mul(
                                o_sb[pb:pb + 64, tt, :],
                                pbc[:], norm_state["ou"][r][pb:pb + 64, :])

                # skewed pipeline: exp of pair 0 starts before the (long) v
                # projection occupies the PE; attention of pair t overlaps
                # k/q projections of pair t+2 and S/exp of pair t+1.
                wq0 = w_pool.tile([P, CO, P], F16, tag="wqk")
                nc.sync.dma_start(wq0[:], wq_pk[0])
                wk0 = w_pool.tile([P, CO, P], F16, tag="wqk")
                nc.sync.dma_start(wk0[:], wk_pk[0])
                for co in range(1, CO):
                    nc.sync.dma_start(xT[:, co, :], xbT_t[:, co, :])
                kq_proj(0, wk=wk0, wq=wq0)
                Es = {0: s_exp(0)}
                kq_proj(1)
                Es[1] = s_exp(1)
                kq_proj(2)

                # ---- v projection: [m, e] ----
                for vh in range(2):
                    wv = w_pool.tile([P, CO, NQ], F16, tag="wv", name=f"wv{vh}")
                    nc.sync.dma_start(wv[:], wv_pk[vh])
                    for mo in range(MO):
                        pv = ps_proj.tile([P, NQ], F32, tag="pp",
                                          name=f"pv{vh}_{mo}")
                        for co in range(CO):
                            nc.tensor.matmul(
                                pv[:], xT[:, co, mo * P:(mo + 1) * P],
                                wv[:, co, :],
                                start=(co == 0), stop=(co == CO - 1))
                        nc.vector.tensor_scalar_mul(
                            v_sb[:, mo, vh * 8:(vh + 1) * 8, 0:D],
                            pv[:].rearrange("p (h d) -> p h d", d=D),
                            mask_sb[:, mo:mo + 1])
                for mo in range(MO):
                    nc.vector.tensor_scalar_mul(
                        v_sb[:, mo, :, D], onesH[:], mask_sb[:, mo:mo + 1])

                for t in range(H // 2):
                    if t + 2 < H // 2:
                        Es[t + 2] = s_exp(t + 2)
                    if t + 3 < H // 2:
                        kq_proj(t + 3)
                    if t == 5:  # stage w_out late, off the critical DMA path
                        for eo in range(EO):
                            nc.sync.dma_start(wo[:, eo, :], wo_pk[:, eo, :])
                    av_norm(t, Es.pop(t))

                # ---- output projection + bias ----
                for no in range(NO):
                    for ch in range(2):
                        py = ps_proj.tile([P, NQ], F32, tag="pp",
                                          name=f"py{no}_{ch}")
                        for eo in range(EO):
                            nc.tensor.matmul(
                                py[:], o_sb[:, eo, no * P:(no + 1) * P],
                                wo[:, eo, ch * NQ:(ch + 1) * NQ],
                                start=(eo == 0), stop=False)
                        nc.tensor.matmul(
                            py[:], ones_row[:],
                            bias_sb[:, ch * NQ:(ch + 1) * NQ],
                            start=False, stop=True)
                        ysb = ou_pool.tile([P, NQ], F32, tag="ysb",
                                           name=f"ysb{no}_{ch}")
                        nc.vector.tensor_copy(ysb[:], py[:])
                        nc.sync.dma_start(
                            y_t[:, no, ch * NQ:(ch + 1) * NQ], ysb[:])

    nc.finalize()
    return nc


_NC_CACHE = None


def _get_nc():
    global _NC_CACHE
    if _NC_CACHE is None:
        _NC_CACHE = build_nc()
    return _NC_CACHE


def _make_in_maps(x, mask, w_qkv, w_out, b_out):
    x = np.ascontiguousarray(np.asarray(x), dtype=np.float32)
    mask_f = np.asarray(mask).astype(np.float32)
    wqkv_h = np.asarray(w_qkv).astype(np.float16)
    wout_h = np.asarray(w_out).astype(np.float16)
    bout_h = np.asarray(b_out).astype(np.float16)
    # pack weights so every device DMA is contiguous per partition:
    # w_qkv [C, 3HD] -> per head pair t: [p, co, ecols] with C = (co p)
    wq4 = wqkv_h.reshape(CO, P, 3 * H * D)
    wq_pk = np.ascontiguousarray(
        wq4[:, :, 0:C].reshape(CO, P, H // 2, P).transpose(2, 1, 0, 3))
    wk_pk = np.ascontiguousarray(
        wq4[:, :, C:2 * C].reshape(CO, P, H // 2, P).transpose(2, 1, 0, 3))
    wv_pk = np.ascontiguousarray(
        wq4[:, :, 2 * C:].reshape(CO, P, 2, NQ).transpose(2, 1, 0, 3))
    wo_pk = np.ascontiguousarray(
        wout_h.reshape(EO, P, C).transpose(1, 0, 2))
    in_maps = []
    for i in range(N_CORES):
        b, q0 = i // 2, (i % 2) * NQ
        xbT = np.ascontiguousarray(np.roll(x[b], -q0, axis=0).T.astype(np.float16))
        mb = np.ascontiguousarray(np.roll(mask_f[b], -q0))
        in_maps.append({"xbT": xbT, "maskb": mb, "wq_pk": wq_pk,
                        "wk_pk": wk_pk, "wv_pk": wv_pk, "wo_pk": wo_pk,
                        "b_out": bout_h})
    return in_maps


def run_kernel(x, mask, w_qkv, w_out, b_out, trace=False):
    """Run on 8 cores; returns (full output [B,N,C], BassKernelResults)."""
    nc = _get_nc()
    in_maps = _make_in_maps(x, mask, w_qkv, w_out, b_out)
    res = run_bass_kernel_spmd(nc, in_maps, core_ids=list(range(N_CORES)),
                               trace=trace)
    out = np.empty((B, N, C), dtype=np.float32)
    for i in range(N_CORES):
        b, q0 = i // 2, (i % 2) * NQ
        out[b, q0:q0 + NQ, :] = res.results[i]["y"]
    return out, res


def kernel(x, mask, w_qkv, w_out, b_out):
    os.environ.setdefault("BASS_NEVER_TRACE", "1")
    out, _ = run_kernel(x, mask, w_qkv, w_out, b_out, trace=False)
    return out



# revision 2
# speedup vs baseline: 1.1668x; 1.1668x over previous
"""Multi-head attention (B=4, N=1024, C=1024, H=16, D=64) on 8 Trainium2 cores.

Sharding: tensor-parallel over heads (the spec hint). Core i handles batch
b = i//2 and head group hg = i%2 (heads 8*hg..8*hg+7): it projects q/k/v for
its 8 heads over all 1024 rows, runs attention, and computes a PARTIAL output
projection over its 512 e-dims (w_out rows 512*hg..+512). The host sums the
two partials of each batch (the tensor-parallel all-reduce) and adds b_out.
No redundant compute: 6.45 GFLOP/core (vs 8.6 for query-parallel with
duplicated k/v).

Matmuls run in fp16 (1 PE cycle/row, FWL weight loads), accumulation fp32 in
PSUM. exp(s/8) stays in fp16 range without a bias (scores ~N(0,1), max ~6;
unnormalized out^T < ~1e3 << 65504). Softmax denominators come from an
appended ones-column in v (masked), reciprocal via the fast approx DVE op
(~5x faster than nc.vector.reciprocal, 51 ULP).

Per-core pipeline (x^T prepared on host):
  1. Per head pair t (4 pairs): k^T/q^T column projections ([128e, 1024]).
  2. S^T per (pair, mo): K=64 matmuls, heads alternate PE row groups 0/64;
     exp on ACT -> E [128m, mo, j, n] fp16. ACT is co-critical (~64us): the
     pipeline keeps S(t+1)/AV(t)/projections on the PE underneath it.
  3. AV per (head, nh): out^T (unnormalized) + denominator via ones col.
     Per-pair normalize: approx-reciprocal -> K=1 ones-matmul partition
     broadcast (row groups 0/32 concurrent) -> DVE multiply into o_sb.
  4. v projection and w_out staging are filler PE/DMA work under exp.
  5. y_partial = o^T.T @ w_out -> fp16 -> DMA out.
"""

import os

import numpy as np

import concourse.bacc as bacc
import concourse.mybir as mybir
import concourse.tile as tile
from concourse.bass_utils import run_bass_kernel_spmd

F32 = mybir.dt.float32
F16 = mybir.dt.float16

B, N, C = 4, 1024, 1024
H, D = 16, 64
HL = 8            # heads per core
P = 128
MO = N // P       # 8 key m-tiles
CO = C // P       # 8 contraction tiles
EO = HL * D // P  # 4 e-tiles (local)
NB = N // P       # 8 output row tiles
NQ2 = 512         # psum free-dim tile
NPAIR = HL // 2   # 4 local head pairs
ATT_SCALE = D ** -0.5
N_CORES = 8


def build_nc():
    nc = bacc.Bacc()
    xbT = nc.declare_dram_parameter("xbT", [C, N], F16, isOutput=False)
    maskb = nc.declare_dram_parameter("maskb", [N], F32, isOutput=False)
    wq_pk = nc.declare_dram_parameter("wq_pk", [NPAIR, P, CO, P], F16,
                                      isOutput=False)
    wk_pk = nc.declare_dram_parameter("wk_pk", [NPAIR, P, CO, P], F16,
                                      isOutput=False)
    wv_pk = nc.declare_dram_parameter("wv_pk", [P, CO, HL * D], F16,
                                      isOutput=False)
    wo_pk = nc.declare_dram_parameter("wo_pk", [P, EO, C], F16, isOutput=False)
    y = nc.declare_dram_parameter("y", [N, C], F16, isOutput=True)

    xbT_t = xbT.rearrange("(co p) m -> p co m", p=P)
    y_t = y.rearrange("(nb p) c -> p nb c", p=P)

    with tile.TileContext(nc) as tc:
        with tc.tile_pool(name="consts", bufs=1) as consts, \
             tc.tile_pool(name="persist", bufs=1) as persist:
            # ---- constants ----
            ones_q = consts.tile([33, D], F16)       # K=1 broadcast lhsT
            nc.vector.memset(ones_q[:], 1.0)
            onesH = consts.tile([P, HL], F16)
            nc.vector.memset(onesH[:], 1.0)
            mask_sb = consts.tile([P, MO], F32)
            nc.sync.dma_start(mask_sb[:], maskb.rearrange("(o p) -> p o", p=P))

            # ---- persistent tensors ----
            qT = persist.tile([P, NPAIR, N], F16)        # q^T: [e, n]
            kT = persist.tile([P, NPAIR, N], F16)        # k^T: [e, m]
            v_sb = persist.tile([P, MO, HL, D + 1], F16)  # v + ones col
            o_sb = persist.tile([P, EO, N], F16)         # normalized out^T
            wo = persist.tile([P, EO, C], F16)           # w_out slice

            with tc.tile_pool(name="xT_pool", bufs=1) as xT_pool, \
                 tc.tile_pool(name="w_pool", bufs=6) as w_pool, \
                 tc.tile_pool(name="wv_pool", bufs=1) as wv_pool, \
                 tc.tile_pool(name="E_pool", bufs=2) as E_pool, \
                 tc.tile_pool(name="den_pool", bufs=2) as den_pool, \
                 tc.tile_pool(name="rcp_pool", bufs=2) as rcp_pool, \
                 tc.tile_pool(name="ou_pool", bufs=2) as ou_pool, \
                 tc.tile_pool(name="ys_pool", bufs=4) as ys_pool, \
                 tc.tile_pool(name="ps_proj", bufs=2, space="PSUM") as ps_proj, \
                 tc.tile_pool(name="ps_s", bufs=1, space="PSUM") as ps_s, \
                 tc.tile_pool(name="ps_av", bufs=2, space="PSUM") as ps_av:
                xT = xT_pool.tile([P, CO, N], F16)       # x^T: [c, m]
                nc.sync.dma_start(xT[:, 0, :], xbT_t[:, 0, :])

                def kq_proj(t, wk=None, wq=None):
                    if wk is None:
                        wk = w_pool.tile([P, CO, P], F16, tag="wqk",
                                         name=f"wk{t}")
                        nc.sync.dma_start(wk[:], wk_pk[t])
                    for nh in range(2):
                        pk = ps_proj.tile([P, NQ2], F32, tag="pp",
                                          name=f"pk{t}_{nh}")
                        for co in range(CO):
                            nc.tensor.matmul(
                                pk[:], wk[:, co, :],
                                xT[:, co, nh * NQ2:(nh + 1) * NQ2],
                                start=(co == 0), stop=(co == CO - 1))
                        nc.vector.tensor_copy(
                            kT[:, t, nh * NQ2:(nh + 1) * NQ2], pk[:])
                    if wq is None:
                        wq = w_pool.tile([P, CO, P], F16, tag="wqk",
                                         name=f"wq{t}")
                        nc.sync.dma_start(wq[:], wq_pk[t])
                    for nh in range(2):
                        pq = ps_proj.tile([P, NQ2], F32, tag="pp",
                                          name=f"pq{t}_{nh}")
                        for co in range(CO):
                            nc.tensor.matmul(
                                pq[:], wq[:, co, :],
                                xT[:, co, nh * NQ2:(nh + 1) * NQ2],
                                start=(co == 0), stop=(co == CO - 1))
                        nc.vector.tensor_copy(
                            qT[:, t, nh * NQ2:(nh + 1) * NQ2], pq[:])

                def s_exp(t):
                    # S^T + exp, one [128, 2048] psum per mo: 2 heads (row
                    # groups 0/64 overlap) x 2 n-halves, then one ACT call.
                    E_pair = E_pool.tile([P, MO, 2, N], F16, tag="E",
                                         name=f"E{t}")
                    for mo in range(MO):
                        pss = ps_s.tile([P, 4 * NQ2], F32, tag="ps_s",
                                        name=f"pss{t}_{mo}")
                        for nh in range(2):
                            for j in range(2):
                                pb = 64 * j
                                nc.tensor.matmul(
                                    pss[:, (2 * j + nh) * NQ2:
                                        (2 * j + nh + 1) * NQ2],
                                    kT[pb:pb + 64, t, mo * P:(mo + 1) * P],
                                    qT[pb:pb + 64, t,
                                       nh * NQ2:(nh + 1) * NQ2],
                                    start=True, stop=True)
                        nc.scalar.activation(
                            E_pair[:, mo, :, :], pss[:],
                            mybir.ActivationFunctionType.Exp,
                            scale=ATT_SCALE)
                    return E_pair

                def av_norm(t, E_pair):
                    # AV (unnormalized) + denominator, then per-pair norm.
                    o_un = ou_pool.tile([P, N], F16, tag="ou", name=f"ou{t}")
                    den = den_pool.tile([33, N], F32, tag="den",
                                        name=f"den{t}")
                    for j in range(2):
                        h = 2 * t + j
                        pb = 64 * j
                        for nh in range(2):
                            pav = ps_av.tile([P, NQ2], F32, tag="ps_av",
                                             name=f"pav{h}_{nh}")
                            for mo in range(MO):
                                nc.tensor.matmul(
                                    pav[0:D + 1, :], v_sb[:, mo, h, :],
                                    E_pair[:, mo, j,
                                           nh * NQ2:(nh + 1) * NQ2],
                                    start=(mo == 0), stop=(mo == MO - 1))
                            nc.vector.tensor_copy(
                                o_un[pb:pb + 64, nh * NQ2:(nh + 1) * NQ2],
                                pav[0:D, :])
                            nc.vector.tensor_copy(
                                den[32 * j:32 * j + 1,
                                    nh * NQ2:(nh + 1) * NQ2],
                                pav[D:D + 1, :])
                    rcp32 = rcp_pool.tile([33, N], F32, tag="rcp32",
                                          name=f"rcp32_{t}")
                    nc.vector.reciprocal_approx_fast(rcp32[:], den[:])
                    rcp16 = rcp_pool.tile([33, N], F16, tag="rcp16",
                                          name=f"rcp16_{t}")
                    nc.vector.tensor_copy(rcp16[:], rcp32[:])
                    for nh in range(2):
                        for j in range(2):
                            pb = 64 * j
                            pbc = ps_av.tile([64, NQ2], F32, tag="ps_av",
                                             name=f"pbc{t}_{j}_{nh}")
                            nc.tensor.matmul(
                                pbc[:], ones_q[32 * j:32 * j + 1, :],
                                rcp16[32 * j:32 * j + 1,
                                      nh * NQ2:(nh + 1) * NQ2],
                                start=True, stop=True,
                                tile_position=(32 * j, 0))
                            nc.vector.tensor_mul(
                                o_sb[pb:pb + 64, t, nh * NQ2:(nh + 1) * NQ2],
                                pbc[:],
                                o_un[pb:pb + 64, nh * NQ2:(nh + 1) * NQ2])

                # ---- skewed pipeline ----
                wq0 = w_pool.tile([P, CO, P], F16, tag="wqk")
                nc.sync.dma_start(wq0[:], wq_pk[0])
                wk0 = w_pool.tile([P, CO, P], F16, tag="wqk")
                nc.sync.dma_start(wk0[:], wk_pk[0])
                for co in range(1, CO):
                    nc.sync.dma_start(xT[:, co, :], xbT_t[:, co, :])
                wv = wv_pool.tile([P, CO, HL * D], F16)
                nc.sync.dma_start(wv[:], wv_pk[:])

                kq_proj(0, wk=wk0, wq=wq0)
                Es = {0: s_exp(0)}
                kq_proj(1)
                Es[1] = s_exp(1)
                kq_proj(2)

                # ---- v projection: [m, e] ----
                for mo in range(MO):
                    pv = ps_proj.tile([P, NQ2], F32, tag="pp",
                                      name=f"pv{mo}")
                    for co in range(CO):
                        nc.tensor.matmul(
                            pv[:], xT[:, co, mo * P:(mo + 1) * P],
                            wv[:, co, :],
                            start=(co == 0), stop=(co == CO - 1))
                    nc.vector.tensor_scalar_mul(
                        v_sb[:, mo, :, 0:D],
                        pv[:].rearrange("p (h d) -> p h d", d=D),
                        mask_sb[:, mo:mo + 1])
                for mo in range(MO):
                    nc.vector.tensor_scalar_mul(
                        v_sb[:, mo, :, D], onesH[:], mask_sb[:, mo:mo + 1])

                for t in range(NPAIR):
                    av_norm(t, Es.pop(t))
                    if t + 2 < NPAIR:
                        Es[t + 2] = s_exp(t + 2)
                    if t + 3 < NPAIR:
                        kq_proj(t + 3)
                    if t == 1:  # stage w_out off the critical DMA path
                        for eo in range(EO):
                            nc.sync.dma_start(wo[:, eo, :], wo_pk[:, eo, :])

                # ---- partial output projection (no bias: host adds) ----
                for nb in range(NB):
                    for ch in range(2):
                        py = ps_proj.tile([P, NQ2], F32, tag="pp",
                                          name=f"py{nb}_{ch}")
                        for eo in range(EO):
                            nc.tensor.matmul(
                                py[:], o_sb[:, eo, nb * P:(nb + 1) * P],
                                wo[:, eo, ch * NQ2:(ch + 1) * NQ2],
                                start=(eo == 0), stop=(eo == EO - 1))
                        ysb = ys_pool.tile([P, NQ2], F16, tag="ysb",
                                           name=f"ysb{nb}_{ch}")
                        nc.vector.tensor_copy(ysb[:], py[:])
                        nc.sync.dma_start(
                            y_t[:, nb, ch * NQ2:(ch + 1) * NQ2], ysb[:])

    nc.finalize()
    return nc


_NC_CACHE = None


def _get_nc():
    global _NC_CACHE
    if _NC_CACHE is None:
        _NC_CACHE = build_nc()
    return _NC_CACHE


def _make_in_maps(x, mask, w_qkv, w_out, b_out):
    x = np.ascontiguousarray(np.asarray(x), dtype=np.float32)
    mask_f = np.asarray(mask).astype(np.float32)
    wqkv_h = np.asarray(w_qkv).astype(np.float16)
    wout_h = np.asarray(w_out).astype(np.float16)
    # per head group hg: q cols hg*512..+512 of wqkv[:, 0:C], k of [C:2C],
    # v of [2C:3C]; w_out rows hg*512..+512. Packed so every DMA is
    # contiguous per partition: C = (co p).
    wq4 = wqkv_h.reshape(CO, P, 3 * H * D)
    xT_h = [np.ascontiguousarray(x[b].T.astype(np.float16)) for b in range(B)]
    in_maps = []
    for i in range(N_CORES):
        b, hg = i // 2, i % 2
        s = hg * HL * D
        wq_pk = np.ascontiguousarray(
            wq4[:, :, s:s + 512].reshape(CO, P, NPAIR, P)
            .transpose(2, 1, 0, 3))
        wk_pk = np.ascontiguousarray(
            wq4[:, :, C + s:C + s + 512].reshape(CO, P, NPAIR, P)
            .transpose(2, 1, 0, 3))
        wv_pk = np.ascontiguousarray(
            wq4[:, :, 2 * C + s:2 * C + s + 512].transpose(1, 0, 2))
        wo_pk = np.ascontiguousarray(
            wout_h[s:s + 512, :].reshape(EO, P, C).transpose(1, 0, 2))
        in_maps.append({"xbT": xT_h[b], "maskb": mask_f[b],
                        "wq_pk": wq_pk, "wk_pk": wk_pk, "wv_pk": wv_pk,
                        "wo_pk": wo_pk})
    return in_maps


def run_kernel(x, mask, w_qkv, w_out, b_out, trace=False):
    """Run on 8 cores; returns (full output [B,N,C], BassKernelResults)."""
    nc = _get_nc()
    in_maps = _make_in_maps(x, mask, w_qkv, w_out, b_out)
    res = run_bass_kernel_spmd(nc, in_maps, core_ids=list(range(N_CORES)),
                               trace=trace)
    bout_f = np.asarray(b_out).astype(np.float32)
    out = np.empty((B, N, C), dtype=np.float32)
    for b in range(B):
        out[b] = (res.results[2 * b]["y"].astype(np.float32)
                  + res.results[2 * b + 1]["y"].astype(np.float32) + bout_f)
    return out, res


def kernel(x, mask, w_qkv, w_out, b_out):
    os.environ.setdefault("BASS_NEVER_TRACE", "1")
    out, _ = run_kernel(x, mask, w_qkv, w_out, b_out, trace=False)
    return out


# revision 7
# speedup vs baseline: 1.3738x; 1.1774x over previous
"""Multi-head attention (B=4, N=1024, C=1024, H=16, D=64) on 8 Trainium2 cores.

Sharding: tensor-parallel over heads (the spec hint). Core i handles batch
b = i//2 and head group hg = i%2 (heads 8*hg..8*hg+7): it projects q/k/v for
its 8 heads over all 1024 rows, runs attention, and computes PARTIAL output
projections over its 512 e-dims (w_out rows 512*hg..+512), split into two
eo-halves (y_a: pairs 0-1, y_b: pairs 2-3) so most of the output projection
runs before the last softmax finishes. The host sums the four partials per
batch (the tensor-parallel all-reduce) and adds b_out. 6.45 GFLOP/core.

The kernel is explicitly software-pipelined at mo (key-tile) granularity.
The scalar engine's exp chain (32 x [128,2048] ACTIVATEs ~= 76us) and the
single 4-bank score psum force strict S(mo) -> exp(mo) -> S(mo+1)
alternation; every other PE unit (projections, AV, normalize-broadcast,
output partials) is interleaved between S groups so the PE never idles and
the HAM clock never re-throttles. Dummy warmup matmuls run during the input
DMA so the first real matmul is already at 2.4 GHz. Non-urgent input DMAs go
through gpsimd SWDGE to keep the sync HWDGE queue short for the
critical-path loads.
"""

import os

import numpy as np

import concourse.bacc as bacc
import concourse.mybir as mybir
import concourse.tile as tile
from concourse.bass_utils import run_bass_kernel_spmd

F32 = mybir.dt.float32
F16 = mybir.dt.float16

B, N, C = 4, 1024, 1024
H, D = 16, 64
HL = 8            # heads per core
P = 128
MO = N // P       # 8 key m-tiles
CO = C // P       # 8 contraction tiles
EO = HL * D // P  # 4 e-tiles (local)
NB = N // P       # 8 output row tiles
NQ2 = 512         # psum free-dim tile
NPAIR = HL // 2   # 4 local head pairs
ATT_SCALE = D ** -0.5
N_CORES = 8


def build_nc():
    nc = bacc.Bacc()
    xbT = nc.declare_dram_parameter("xbT", [C, N], F16, isOutput=False)
    maskb = nc.declare_dram_parameter("maskb", [N], F32, isOutput=False)
    wq_pk = nc.declare_dram_parameter("wq_pk", [NPAIR, P, CO, P], F16,
                                      isOutput=False)
    wk_pk = nc.declare_dram_parameter("wk_pk", [NPAIR, P, CO, P], F16,
                                      isOutput=False)
    wv_pk = nc.declare_dram_parameter("wv_pk", [P, CO, HL * D], F16,
                                      isOutput=False)
    wo_pk = nc.declare_dram_parameter("wo_pk", [P, EO, C], F16, isOutput=False)
    y_a = nc.declare_dram_parameter("y_a", [N, C], F16, isOutput=True)
    y_b = nc.declare_dram_parameter("y_b", [N, C], F16, isOutput=True)

    xbT_t = xbT.rearrange("(co p) m -> p co m", p=P)
    ya_t = y_a.rearrange("(nb p) c -> p nb c", p=P)
    yb_t = y_b.rearrange("(nb p) c -> p nb c", p=P)

    with tile.TileContext(nc) as tc:
        with tc.tile_pool(name="consts", bufs=1) as consts, \
             tc.tile_pool(name="persist", bufs=1) as persist:
            ones_q = consts.tile([33, D], F16)       # K=1 broadcast lhsT
            nc.vector.memset(ones_q[:], 1.0)
            ones_row = consts.tile([1, NQ2], F16)    # warmup rhs
            nc.vector.memset(ones_row[:], 1.0)
            onesH = consts.tile([P, HL], F16)
            nc.vector.memset(onesH[:], 1.0)
            mask_sb = consts.tile([P, MO], F32)

            qT = persist.tile([P, NPAIR, N], F16)
            kT = persist.tile([P, NPAIR, N], F16)
            v_sb = persist.tile([P, MO, HL, D + 1], F16)
            o_sb = persist.tile([P, EO, N], F16)
            wo = persist.tile([P, EO, C], F16)

            with tc.tile_pool(name="xT_pool", bufs=1) as xT_pool, \
                 tc.tile_pool(name="w_pool", bufs=4) as w_pool, \
                 tc.tile_pool(name="wv_pool", bufs=1) as wv_pool, \
                 tc.tile_pool(name="E_pool", bufs=3) as E_pool, \
                 tc.tile_pool(name="den_pool", bufs=2) as den_pool, \
                 tc.tile_pool(name="rcp_pool", bufs=2) as rcp_pool, \
                 tc.tile_pool(name="ou_pool", bufs=2) as ou_pool, \
                 tc.tile_pool(name="ys_pool", bufs=3) as ys_pool, \
                 tc.tile_pool(name="ps_proj", bufs=2, space="PSUM") as ps_proj, \
                 tc.tile_pool(name="ps_s", bufs=1, space="PSUM") as ps_s, \
                 tc.tile_pool(name="ps_av", bufs=2, space="PSUM") as ps_av:
                xT = xT_pool.tile([P, CO, N], F16)

                # ---- HAM warmup: PE busy during input DMA ----
                for i in range(8):
                    pw = ps_av.tile([64, NQ2], F32, tag="av", name=f"warm{i}")
                    nc.tensor.matmul(pw[:], ones_q[0:1, :], ones_row[:],
                                     start=True, stop=True)

                # ---- critical-path DMAs on sync HWDGE ----
                wk0 = w_pool.tile([P, CO, P], F16, tag="wqk", name="wk0")
                nc.sync.dma_start(wk0[:], wk_pk[0])
                nc.sync.dma_start(xT[:, 0, :], xbT_t[:, 0, :])
                wq0 = w_pool.tile([P, CO, P], F16, tag="wqk", name="wq0")
                nc.sync.dma_start(wq0[:], wq_pk[0])
                for co in range(1, CO):
                    nc.sync.dma_start(xT[:, co, :], xbT_t[:, co, :])
                # ---- bulk prefetch (after critical path on sync queue) ----
                nc.sync.dma_start(mask_sb[:],
                                  maskb.rearrange("(o p) -> p o", p=P))
                wv = wv_pool.tile([P, CO, HL * D], F16)
                nc.sync.dma_start(wv[:], wv_pk[:])

                wtiles = {0: (wk0, wq0)}

                def prefetch_w(t):
                    wk = w_pool.tile([P, CO, P], F16, tag="wqk",
                                     name=f"wk{t}")
                    nc.sync.dma_start(wk[:], wk_pk[t])
                    wq = w_pool.tile([P, CO, P], F16, tag="wqk",
                                     name=f"wq{t}")
                    nc.sync.dma_start(wq[:], wq_pk[t])
                    wtiles[t] = (wk, wq)

                def emit_proj(t, kind, nh):
                    # one [128, 512] psum group of the k/q projection
                    w = wtiles[t][0 if kind == "k" else 1]
                    dst = kT if kind == "k" else qT
                    pp = ps_proj.tile([P, NQ2], F32, tag="pp",
                                      name=f"p{kind}{t}_{nh}")
                    for co in range(CO):
                        nc.tensor.matmul(
                            pp[:], w[:, co, :],
                            xT[:, co, nh * NQ2:(nh + 1) * NQ2],
                            start=(co == 0), stop=(co == CO - 1))
                    nc.vector.tensor_copy(
                        dst[:, t, nh * NQ2:(nh + 1) * NQ2], pp[:])

                def emit_v(mo):
                    pv = ps_proj.tile([P, NQ2], F32, tag="pp",
                                      name=f"pv{mo}")
                    for co in range(CO):
                        nc.tensor.matmul(
                            pv[:], xT[:, co, mo * P:(mo + 1) * P],
                            wv[:, co, :],
                            start=(co == 0), stop=(co == CO - 1))
                    nc.vector.tensor_scalar_mul(
                        v_sb[:, mo, :, 0:D],
                        pv[:].rearrange("p (h d) -> p h d", d=D),
                        mask_sb[:, mo:mo + 1])
                    nc.vector.tensor_scalar_mul(
                        v_sb[:, mo, :, D], onesH[:], mask_sb[:, mo:mo + 1])

                def emit_s(t, mo, E_pair):
                    pss = ps_s.tile([P, 4 * NQ2], F32, tag="ps_s",
                                    name=f"pss{t}_{mo}")
                    for nh in range(2):
                        for j in range(2):
                            pb = 64 * j
                            nc.tensor.matmul(
                                pss[:, (2 * j + nh) * NQ2:
                                    (2 * j + nh + 1) * NQ2],
                                kT[pb:pb + 64, t, mo * P:(mo + 1) * P],
                                qT[pb:pb + 64, t, nh * NQ2:(nh + 1) * NQ2],
                                start=True, stop=True)
                    nc.scalar.activation(
                        E_pair[:, mo, :, :], pss[:],
                        mybir.ActivationFunctionType.Exp, scale=ATT_SCALE)

                norm_state = {}

                def emit_avc(t, j, nh, E_pair):
                    # one AV accumulation chain (one head, one n-half)
                    if t not in norm_state:
                        o_un = ou_pool.tile([P, N], F16, tag="ou",
                                            name=f"ou{t}")
                        den = den_pool.tile([33, N], F32, tag="den",
                                            name=f"den{t}")
                        norm_state[t] = (o_un, den)
                    o_un, den = norm_state[t]
                    h = 2 * t + j
                    pb = 64 * j
                    pav = ps_av.tile([P, NQ2], F32, tag="av",
                                     name=f"pav{h}_{nh}")
                    for mo in range(MO):
                        nc.tensor.matmul(
                            pav[0:D + 1, :], v_sb[:, mo, h, :],
                            E_pair[:, mo, j, nh * NQ2:(nh + 1) * NQ2],
                            start=(mo == 0), stop=(mo == MO - 1))
                    nc.vector.tensor_copy(
                        o_un[pb:pb + 64, nh * NQ2:(nh + 1) * NQ2],
                        pav[0:D, :])
                    nc.vector.tensor_copy(
                        den[32 * j:32 * j + 1, nh * NQ2:(nh + 1) * NQ2],
                        pav[D:D + 1, :])

                def emit_norm(t):
                    o_un, den = norm_state[t]
                    rcp32 = rcp_pool.tile([33, N], F32, tag="rcp32",
                                          name=f"rcp32_{t}")
                    nc.vector.reciprocal_approx_fast(rcp32[:], den[:])
                    rcp16 = rcp_pool.tile([33, N], F16, tag="rcp16",
                                          name=f"rcp16_{t}")
                    nc.vector.tensor_copy(rcp16[:], rcp32[:])
                    for nh in range(2):
                        for j in range(2):
                            pb = 64 * j
                            pbc = ps_proj.tile([64, NQ2], F32, tag="pp",
                                               name=f"pbc{t}_{j}_{nh}")
                            nc.tensor.matmul(
                                pbc[:], ones_q[32 * j:32 * j + 1, :],
                                rcp16[32 * j:32 * j + 1,
                                      nh * NQ2:(nh + 1) * NQ2],
                                start=True, stop=True,
                                tile_position=(32 * j, 0))
                            nc.vector.tensor_mul(
                                o_sb[pb:pb + 64, t, nh * NQ2:(nh + 1) * NQ2],
                                pbc[:],
                                o_un[pb:pb + 64, nh * NQ2:(nh + 1) * NQ2])

                def emit_out(part, nb, ch):
                    # partial output projection over eo pair `part`
                    yt = ya_t if part == 0 else yb_t
                    py = ps_proj.tile([P, NQ2], F32, tag="pp",
                                      name=f"py{part}_{nb}_{ch}")
                    for ei in range(2):
                        eo = 2 * part + ei
                        nc.tensor.matmul(
                            py[:], o_sb[:, eo, nb * P:(nb + 1) * P],
                            wo[:, eo, ch * NQ2:(ch + 1) * NQ2],
                            start=(ei == 0), stop=(ei == 1))
                    ysb = ys_pool.tile([P, NQ2], F16, tag="ysb",
                                       name=f"ysb{part}_{nb}_{ch}")
                    nc.vector.tensor_copy(ysb[:], py[:])
                    nc.sync.dma_start(
                        yt[:, nb, ch * NQ2:(ch + 1) * NQ2], ysb[:])

                def fetch_wo():
                    for eo in range(EO):
                        nc.sync.dma_start(wo[:, eo, :], wo_pk[:, eo, :])

                # ---- prologue: minimal work before the first exp ----
                emit_proj(0, "k", 0)
                emit_proj(0, "q", 0)
                emit_proj(0, "q", 1)

                Es = {t: E_pool.tile([P, MO, 2, N], F16, tag="E",
                                     name=f"E{t}")
                      for t in range(NPAIR)}

                # fillers emitted after each S(t, mo) group; deadlines:
                # kq(t) before S(t,0); k_nh1(0) before S(0,4); v before AV0;
                # AV(t) before exp(t+3) (E bufs=3); norm(t) before out parts.
                F = {
                    (0, 0): [lambda: emit_proj(0, "k", 1)],
                    (0, 1): [lambda: prefetch_w(1), lambda: emit_v(0)],
                    (0, 2): [lambda: emit_v(1)],
                    (0, 3): [lambda: emit_v(2)],
                    (0, 4): [lambda: emit_proj(1, "k", 0)],
                    (0, 5): [lambda: emit_proj(1, "k", 1)],
                    (0, 6): [lambda: emit_proj(1, "q", 0)],
                    (0, 7): [lambda: emit_proj(1, "q", 1)],
                    (1, 0): [fetch_wo, lambda: emit_v(3)],
                    (1, 1): [lambda: emit_v(4)],
                    (1, 2): [lambda: emit_v(5)],
                    (1, 3): [lambda: prefetch_w(2), lambda: emit_v(6)],
                    (1, 4): [lambda: emit_proj(2, "k", 0)],
                    (1, 5): [lambda: emit_proj(2, "k", 1)],
                    (1, 6): [lambda: emit_proj(2, "q", 0)],
                    (1, 7): [lambda: emit_proj(2, "q", 1)],
                    (2, 0): [lambda: emit_v(7)],
                    (2, 1): [lambda: emit_avc(0, 0, 0, Es[0])],
                    (2, 2): [lambda: emit_avc(0, 0, 1, Es[0])],
                    (2, 3): [lambda: prefetch_w(3),
                             lambda: emit_avc(0, 1, 0, Es[0])],
                    (2, 4): [lambda: emit_proj(3, "k", 0),
                             lambda: emit_avc(0, 1, 1, Es[0])],
                    (2, 5): [lambda: emit_proj(3, "k", 1),
                             lambda: emit_norm(0)],
                    (2, 6): [lambda: emit_proj(3, "q", 0)],
                    (2, 7): [lambda: emit_proj(3, "q", 1)],
                    (3, 0): [lambda: emit_avc(1, 0, 0, Es[1])],
                    (3, 1): [lambda: emit_avc(1, 0, 1, Es[1])],
                    (3, 2): [lambda: emit_avc(1, 1, 0, Es[1]),
                             lambda: emit_avc(1, 1, 1, Es[1])],
                    (3, 3): [lambda: emit_norm(1),
                             lambda: emit_out(0, 0, 0),
                             lambda: emit_out(0, 0, 1)],
                    (3, 4): [lambda: emit_avc(2, 0, 0, Es[2]),
                             lambda: emit_avc(2, 0, 1, Es[2]),
                             lambda: emit_out(0, 1, 0)],
                    (3, 5): [lambda: emit_avc(2, 1, 0, Es[2]),
                             lambda: emit_avc(2, 1, 1, Es[2]),
                             lambda: emit_out(0, 1, 1)],
                    (3, 6): [lambda: emit_norm(2)],
                    (3, 7): [lambda: emit_avc(3, 0, 0, Es[3]),
                             lambda: emit_avc(3, 0, 1, Es[3]),
                             lambda: emit_avc(3, 1, 0, Es[3]),
                             lambda: emit_avc(3, 1, 1, Es[3])],
                }

                for t in range(NPAIR):
                    for mo in range(MO):
                        emit_s(t, mo, Es[t])
                        for f in F.get((t, mo), []):
                            f()

                # ---- tail ----
                emit_norm(3)
                for nb in range(2, NB):
                    emit_out(0, nb, 0)
                    emit_out(0, nb, 1)
                for nb in range(NB):
                    emit_out(1, nb, 0)
                    emit_out(1, nb, 1)

    nc.finalize()
    return nc


_NC_CACHE = None


def _get_nc():
    global _NC_CACHE
    if _NC_CACHE is None:
        _NC_CACHE = build_nc()
    return _NC_CACHE


def _make_in_maps(x, mask, w_qkv, w_out, b_out):
    x = np.ascontiguousarray(np.asarray(x), dtype=np.float32)
    mask_f = np.asarray(mask).astype(np.float32)
    wqkv_h = np.asarray(w_qkv).astype(np.float16)
    wout_h = np.asarray(w_out).astype(np.float16)
    # per head group hg: q cols hg*512..+512 of wqkv[:, 0:C], k of [C:2C],
    # v of [2C:3C]; w_out rows hg*512..+512. Packed so every DMA is
    # contiguous per partition: C = (co p).
    wq4 = wqkv_h.reshape(CO, P, 3 * H * D)
    xT_h = [np.ascontiguousarray(x[b].T.astype(np.float16)) for b in range(B)]
    in_maps = []
    for i in range(N_CORES):
        b, hg = i // 2, i % 2
        s = hg * HL * D
        wq_pk = np.ascontiguousarray(
            wq4[:, :, s:s + 512].reshape(CO, P, NPAIR, P)
            .transpose(2, 1, 0, 3))
        wk_pk = np.ascontiguousarray(
            wq4[:, :, C + s:C + s + 512].reshape(CO, P, NPAIR, P)
            .transpose(2, 1, 0, 3))
        wv_pk = np.ascontiguousarray(
            wq4[:, :, 2 * C + s:2 * C + s + 512].transpose(1, 0, 2))
        wo_pk = np.ascontiguousarray(
            wout_h[s:s + 512, :].reshape(EO, P, C).transpose(1, 0, 2))
        in_maps.append({"xbT": xT_h[b], "maskb": mask_f[b],
                        "wq_pk": wq_pk, "wk_pk": wk_pk, "wv_pk": wv_pk,
                        "wo_pk": wo_pk})
    return in_maps


def run_kernel(x, mask, w_qkv, w_out, b_out, trace=False):
    """Run on 8 cores; returns (full output [B,N,C], BassKernelResults)."""
    nc = _get_nc()
    in_maps = _make_in_maps(x, mask, w_qkv, w_out, b_out)
    res = run_bass_kernel_spmd(nc, in_maps, core_ids=list(range(N_CORES)),
                               trace=trace)
    bout_f = np.asarray(b_out).astype(np.float32)
    out = np.empty((B, N, C), dtype=np.float32)
    for b in range(B):
        acc = bout_f.copy()
        for i in (2 * b, 2 * b + 1):
            acc = acc + res.results[i]["y_a"].astype(np.float32) \
                      + res.results[i]["y_b"].astype(np.float32)
        out[b] = acc
    return out, res


def kernel(x, mask, w_qkv, w_out, b_out):
    os.environ.setdefault("BASS_NEVER_TRACE", "1")
    out, _ = run_kernel(x, mask, w_qkv, w_out, b_out, trace=False)
    return out


# revision 8
# speedup vs baseline: 1.3926x; 1.0137x over previous
"""Multi-head attention (B=4, N=1024, C=1024, H=16, D=64) on 8 Trainium2 cores.

Sharding: tensor-parallel over heads (the spec hint). Core i handles batch
b = i//2 and head group hg = i%2 (heads 8*hg..8*hg+7): it projects q/k/v for
its 8 heads over all 1024 rows, runs attention, and computes PARTIAL output
projections over its 512 e-dims (w_out rows 512*hg..+512), split into two
eo-halves (y_a: pairs 0-1, y_b: pairs 2-3) so most of the output projection
runs before the last softmax finishes. The host sums the four partials per
batch (the tensor-parallel all-reduce) and adds b_out. 6.45 GFLOP/core.

The kernel is explicitly software-pipelined at mo (key-tile) granularity.
The scalar engine's exp chain (32 x [128,2048] ACTIVATEs ~= 76us) and the
single 4-bank score psum force strict S(mo) -> exp(mo) -> S(mo+1)
alternation; every other PE unit (projections, AV, normalize-broadcast,
output partials) is interleaved between S groups so the PE never idles and
the HAM clock never re-throttles. Dummy warmup matmuls run during the input
DMA so the first real matmul is already at 2.4 GHz. Non-urgent input DMAs go
through gpsimd SWDGE to keep the sync HWDGE queue short for the
critical-path loads.
"""

import os

import numpy as np

import concourse.bacc as bacc
import concourse.mybir as mybir
import concourse.tile as tile
from concourse.bass_utils import run_bass_kernel_spmd

F32 = mybir.dt.float32
F16 = mybir.dt.float16

B, N, C = 4, 1024, 1024
H, D = 16, 64
HL = 8            # heads per core
P = 128
MO = N // P       # 8 key m-tiles
CO = C // P       # 8 contraction tiles
EO = HL * D // P  # 4 e-tiles (local)
NB = N // P       # 8 output row tiles
NQ2 = 512         # psum free-dim tile
NPAIR = HL // 2   # 4 local head pairs
ATT_SCALE = D ** -0.5
N_CORES = 8


def build_nc():
    nc = bacc.Bacc()
    xbT = nc.declare_dram_parameter("xbT", [C, N], F16, isOutput=False)
    maskb = nc.declare_dram_parameter("maskb", [N], F32, isOutput=False)
    wq_pk = nc.declare_dram_parameter("wq_pk", [NPAIR, P, CO, P], F16,
                                      isOutput=False)
    wk_pk = nc.declare_dram_parameter("wk_pk", [NPAIR, P, CO, P], F16,
                                      isOutput=False)
    wv_pk = nc.declare_dram_parameter("wv_pk", [P, CO, HL * D], F16,
                                      isOutput=False)
    wo_pk = nc.declare_dram_parameter("wo_pk", [P, EO, C], F16, isOutput=False)
    y_a = nc.declare_dram_parameter("y_a", [N, C], F16, isOutput=True)
    y_b = nc.declare_dram_parameter("y_b", [N, C], F16, isOutput=True)

    xbT_t = xbT.rearrange("(co p) m -> p co m", p=P)
    ya_t = y_a.rearrange("(nb p) c -> p nb c", p=P)
    yb_t = y_b.rearrange("(nb p) c -> p nb c", p=P)

    with tile.TileContext(nc) as tc:
        with tc.tile_pool(name="consts", bufs=1) as consts, \
             tc.tile_pool(name="persist", bufs=1) as persist:
            ones_q = consts.tile([33, D], F16)       # K=1 broadcast lhsT
            nc.vector.memset(ones_q[:], 1.0)
            ones_row = consts.tile([1, NQ2], F16)    # warmup rhs
            nc.vector.memset(ones_row[:], 1.0)
            onesH = consts.tile([P, HL], F16)
            nc.vector.memset(onesH[:], 1.0)
            mask_sb = consts.tile([P, MO], F32)

            qT = persist.tile([P, NPAIR, N], F16)
            kT = persist.tile([P, NPAIR, N], F16)
            v_sb = persist.tile([P, MO, HL, D + 1], F16)
            o_sb = persist.tile([P, EO, N], F16)
            wo = persist.tile([P, EO, C], F16)

            with tc.tile_pool(name="xT_pool", bufs=1) as xT_pool, \
                 tc.tile_pool(name="w_pool", bufs=4) as w_pool, \
                 tc.tile_pool(name="wv_pool", bufs=1) as wv_pool, \
                 tc.tile_pool(name="E_pool", bufs=3) as E_pool, \
                 tc.tile_pool(name="den_pool", bufs=2) as den_pool, \
                 tc.tile_pool(name="rcp_pool", bufs=2) as rcp_pool, \
                 tc.tile_pool(name="ou_pool", bufs=2) as ou_pool, \
                 tc.tile_pool(name="ys_pool", bufs=3) as ys_pool, \
                 tc.tile_pool(name="ps_proj", bufs=2, space="PSUM") as ps_proj, \
                 tc.tile_pool(name="ps_s", bufs=1, space="PSUM") as ps_s, \
                 tc.tile_pool(name="ps_av", bufs=2, space="PSUM") as ps_av:
                xT = xT_pool.tile([P, CO, N], F16)

                # ---- HAM warmup: PE busy during input DMA ----
                for i in range(8):
                    pw = ps_av.tile([64, NQ2], F32, tag="av", name=f"warm{i}")
                    nc.tensor.matmul(pw[:], ones_q[0:1, :], ones_row[:],
                                     start=True, stop=True)

                # ---- critical-path DMAs on sync HWDGE ----
                wk0 = w_pool.tile([P, CO, P], F16, tag="wqk", name="wk0")
                nc.sync.dma_start(wk0[:], wk_pk[0])
                nc.sync.dma_start(xT[:, 0, :], xbT_t[:, 0, :])
                wq0 = w_pool.tile([P, CO, P], F16, tag="wqk", name="wq0")
                nc.sync.dma_start(wq0[:], wq_pk[0])
                for co in range(1, CO):
                    eng = nc.sync if co % 2 == 0 else nc.scalar
                    eng.dma_start(xT[:, co, :], xbT_t[:, co, :])
                # ---- bulk prefetch (split across both HWDGE queues) ----
                nc.sync.dma_start(mask_sb[:],
                                  maskb.rearrange("(o p) -> p o", p=P))
                wv = wv_pool.tile([P, CO, HL * D], F16)
                nc.scalar.dma_start(wv[:], wv_pk[:])

                wtiles = {0: (wk0, wq0)}

                def prefetch_w(t):
                    wk = w_pool.tile([P, CO, P], F16, tag="wqk",
                                     name=f"wk{t}")
                    nc.sync.dma_start(wk[:], wk_pk[t])
                    wq = w_pool.tile([P, CO, P], F16, tag="wqk",
                                     name=f"wq{t}")
                    nc.sync.dma_start(wq[:], wq_pk[t])
                    wtiles[t] = (wk, wq)

                def emit_proj(t, kind, nh):
                    # one [128, 512] psum group of the k/q projection
                    w = wtiles[t][0 if kind == "k" else 1]
                    dst = kT if kind == "k" else qT
                    pp = ps_proj.tile([P, NQ2], F32, tag="pp",
                                      name=f"p{kind}{t}_{nh}")
                    for co in range(CO):
                        nc.tensor.matmul(
                            pp[:], w[:, co, :],
                            xT[:, co, nh * NQ2:(nh + 1) * NQ2],
                            start=(co == 0), stop=(co == CO - 1))
                    nc.vector.tensor_copy(
                        dst[:, t, nh * NQ2:(nh + 1) * NQ2], pp[:])

                def emit_v(mo):
                    pv = ps_proj.tile([P, NQ2], F32, tag="pp",
                                      name=f"pv{mo}")
                    for co in range(CO):
                        nc.tensor.matmul(
                            pv[:], xT[:, co, mo * P:(mo + 1) * P],
                            wv[:, co, :],
                            start=(co == 0), stop=(co == CO - 1))
                    nc.vector.tensor_scalar_mul(
                        v_sb[:, mo, :, 0:D],
                        pv[:].rearrange("p (h d) -> p h d", d=D),
                        mask_sb[:, mo:mo + 1])
                    nc.vector.tensor_scalar_mul(
                        v_sb[:, mo, :, D], onesH[:], mask_sb[:, mo:mo + 1])

                def emit_s(t, mo, E_pair):
                    pss = ps_s.tile([P, 4 * NQ2], F32, tag="ps_s",
                                    name=f"pss{t}_{mo}")
                    for nh in range(2):
                        for j in range(2):
                            pb = 64 * j
                            nc.tensor.matmul(
                                pss[:, (2 * j + nh) * NQ2:
                                    (2 * j + nh + 1) * NQ2],
                                kT[pb:pb + 64, t, mo * P:(mo + 1) * P],
                                qT[pb:pb + 64, t, nh * NQ2:(nh + 1) * NQ2],
                                start=True, stop=True)
                    nc.scalar.activation(
                        E_pair[:, mo, :, :], pss[:],
                        mybir.ActivationFunctionType.Exp, scale=ATT_SCALE)

                norm_state = {}

                def emit_avc(t, j, nh, E_pair):
                    # one AV accumulation chain (one head, one n-half)
                    if t not in norm_state:
                        o_un = ou_pool.tile([P, N], F16, tag="ou",
                                            name=f"ou{t}")
                        den = den_pool.tile([33, N], F32, tag="den",
                                            name=f"den{t}")
                        norm_state[t] = (o_un, den)
                    o_un, den = norm_state[t]
                    h = 2 * t + j
                    pb = 64 * j
                    pav = ps_av.tile([P, NQ2], F32, tag="av",
                                     name=f"pav{h}_{nh}")
                    for mo in range(MO):
                        nc.tensor.matmul(
                            pav[0:D + 1, :], v_sb[:, mo, h, :],
                            E_pair[:, mo, j, nh * NQ2:(nh + 1) * NQ2],
                            start=(mo == 0), stop=(mo == MO - 1))
                    nc.vector.tensor_copy(
                        o_un[pb:pb + 64, nh * NQ2:(nh + 1) * NQ2],
                        pav[0:D, :])
                    nc.vector.tensor_copy(
                        den[32 * j:32 * j + 1, nh * NQ2:(nh + 1) * NQ2],
                        pav[D:D + 1, :])

                def emit_norm_rcp(t):
                    o_un, den = norm_state[t]
                    rcp32 = rcp_pool.tile([33, N], F32, tag="rcp32",
                                          name=f"rcp32_{t}")
                    nc.vector.reciprocal_approx_fast(rcp32[:], den[:])
                    rcp16 = rcp_pool.tile([33, N], F16, tag="rcp16",
                                          name=f"rcp16_{t}")
                    nc.vector.tensor_copy(rcp16[:], rcp32[:])
                    norm_state[t] = (o_un, den, rcp16)

                def emit_norm_bc(t):
                    o_un, den, rcp16 = norm_state[t]
                    for nh in range(2):
                        for j in range(2):
                            pb = 64 * j
                            pbc = ps_proj.tile([64, NQ2], F32, tag="pp",
                                               name=f"pbc{t}_{j}_{nh}")
                            nc.tensor.matmul(
                                pbc[:], ones_q[32 * j:32 * j + 1, :],
                                rcp16[32 * j:32 * j + 1,
                                      nh * NQ2:(nh + 1) * NQ2],
                                start=True, stop=True,
                                tile_position=(32 * j, 0))
                            nc.vector.tensor_mul(
                                o_sb[pb:pb + 64, t, nh * NQ2:(nh + 1) * NQ2],
                                pbc[:],
                                o_un[pb:pb + 64, nh * NQ2:(nh + 1) * NQ2])

                def emit_norm(t):
                    emit_norm_rcp(t)
                    emit_norm_bc(t)

                def emit_out(part, nb, ch, evac="v"):
                    # partial output projection over eo pair `part`
                    yt = ya_t if part == 0 else yb_t
                    py = ps_proj.tile([P, NQ2], F32, tag="pp",
                                      name=f"py{part}_{nb}_{ch}")
                    for ei in range(2):
                        eo = 2 * part + ei
                        nc.tensor.matmul(
                            py[:], o_sb[:, eo, nb * P:(nb + 1) * P],
                            wo[:, eo, ch * NQ2:(ch + 1) * NQ2],
                            start=(ei == 0), stop=(ei == 1))
                    ysb = ys_pool.tile([P, NQ2], F16, tag="ysb",
                                       name=f"ysb{part}_{nb}_{ch}")
                    if evac == "s":
                        nc.scalar.copy(ysb[:], py[:])
                    else:
                        nc.vector.tensor_copy(ysb[:], py[:])
                    nc.sync.dma_start(
                        yt[:, nb, ch * NQ2:(ch + 1) * NQ2], ysb[:])

                def fetch_wo():
                    for eo in range(EO):
                        nc.sync.dma_start(wo[:, eo, :], wo_pk[:, eo, :])

                # ---- prologue: minimal work before the first exp ----
                emit_proj(0, "k", 0)
                emit_proj(0, "q", 0)
                emit_proj(0, "q", 1)

                Es = {t: E_pool.tile([P, MO, 2, N], F16, tag="E",
                                     name=f"E{t}")
                      for t in range(NPAIR)}

                # fillers emitted after each S(t, mo) group; deadlines:
                # kq(t) before S(t,0); k_nh1(0) before S(0,4); v before AV0;
                # AV(t) before exp(t+3) (E bufs=3); norm(t) before out parts.
                F = {
                    (0, 0): [lambda: emit_proj(0, "k", 1)],
                    (0, 1): [lambda: prefetch_w(1), lambda: emit_v(0)],
                    (0, 2): [lambda: emit_v(1)],
                    (0, 3): [lambda: emit_v(2)],
                    (0, 4): [lambda: emit_proj(1, "k", 0)],
                    (0, 5): [lambda: emit_proj(1, "k", 1)],
                    (0, 6): [lambda: emit_proj(1, "q", 0)],
                    (0, 7): [lambda: emit_proj(1, "q", 1)],
                    (1, 0): [fetch_wo, lambda: emit_v(3)],
                    (1, 1): [lambda: emit_v(4)],
                    (1, 2): [lambda: emit_v(5)],
                    (1, 3): [lambda: prefetch_w(2), lambda: emit_v(6)],
                    (1, 4): [lambda: emit_proj(2, "k", 0)],
                    (1, 5): [lambda: emit_proj(2, "k", 1)],
                    (1, 6): [lambda: emit_proj(2, "q", 0)],
                    (1, 7): [lambda: emit_proj(2, "q", 1)],
                    (2, 0): [lambda: emit_v(7),
                             lambda: emit_avc(0, 0, 0, Es[0])],
                    (2, 1): [lambda: emit_avc(0, 0, 1, Es[0]),
                             lambda: emit_avc(0, 1, 0, Es[0])],
                    (2, 2): [lambda: emit_avc(0, 1, 1, Es[0]),
                             lambda: emit_norm_rcp(0)],
                    (2, 3): [lambda: prefetch_w(3),
                             lambda: emit_norm_bc(0),
                             lambda: emit_avc(1, 0, 0, Es[1])],
                    (2, 4): [lambda: emit_proj(3, "k", 0),
                             lambda: emit_avc(1, 0, 1, Es[1])],
                    (2, 5): [lambda: emit_proj(3, "k", 1),
                             lambda: emit_avc(1, 1, 0, Es[1])],
                    (2, 6): [lambda: emit_proj(3, "q", 0),
                             lambda: emit_avc(1, 1, 1, Es[1]),
                             lambda: emit_norm_rcp(1)],
                    (2, 7): [lambda: emit_proj(3, "q", 1),
                             lambda: emit_norm_bc(1)],
                    (3, 0): [lambda: emit_out(0, 0, 0),
                             lambda: emit_out(0, 0, 1),
                             lambda: emit_out(0, 1, 0)],
                    (3, 1): [lambda: emit_out(0, 1, 1),
                             lambda: emit_out(0, 2, 0),
                             lambda: emit_out(0, 2, 1)],
                    (3, 2): [lambda: emit_out(0, 3, 0),
                             lambda: emit_out(0, 3, 1),
                             lambda: emit_out(0, 4, 0)],
                    (3, 3): [lambda: emit_out(0, 4, 1),
                             lambda: emit_out(0, 5, 0),
                             lambda: emit_out(0, 5, 1)],
                    (3, 4): [lambda: emit_avc(2, 0, 0, Es[2]),
                             lambda: emit_avc(2, 0, 1, Es[2]),
                             lambda: emit_out(0, 6, 0)],
                    (3, 5): [lambda: emit_avc(2, 1, 0, Es[2]),
                             lambda: emit_avc(2, 1, 1, Es[2]),
                             lambda: emit_norm_rcp(2),
                             lambda: emit_out(0, 6, 1)],
                    (3, 6): [lambda: emit_norm_bc(2),
                             lambda: emit_out(0, 7, 0),
                             lambda: emit_out(0, 7, 1)],
                    (3, 7): [lambda: emit_avc(3, 0, 0, Es[3]),
                             lambda: emit_avc(3, 0, 1, Es[3]),
                             lambda: emit_avc(3, 1, 0, Es[3]),
                             lambda: emit_avc(3, 1, 1, Es[3]),
                             lambda: emit_norm_rcp(3)],
                }

                for t in range(NPAIR):
                    for mo in range(MO):
                        emit_s(t, mo, Es[t])
                        for f in F.get((t, mo), []):
                            f()

                # ---- tail ----
                emit_norm_bc(3)
                for nb in range(NB):
                    emit_out(1, nb, 0, evac="s")
                    emit_out(1, nb, 1, evac="v")

    nc.finalize()
    return nc


_NC_CACHE = None


def _get_nc():
    global _NC_CACHE
    if _NC_CACHE is None:
        _NC_CACHE = build_nc()
    return _NC_CACHE


def _make_in_maps(x, mask, w_qkv, w_out, b_out):
    x = np.ascontiguousarray(np.asarray(x), dtype=np.float32)
    mask_f = np.asarray(mask).astype(np.float32)
    wqkv_h = np.asarray(w_qkv).astype(np.float16)
    wout_h = np.asarray(w_out).astype(np.float16)
    # per head group hg: q cols hg*512..+512 of wqkv[:, 0:C], k of [C:2C],
    # v of [2C:3C]; w_out rows hg*512..+512. Packed so every DMA is
    # contiguous per partition: C = (co p).
    wq4 = wqkv_h.reshape(CO, P, 3 * H * D)
    xT_h = [np.ascontiguousarray(x[b].T.astype(np.float16)) for b in range(B)]
    in_maps = []
    for i in range(N_CORES):
        b, hg = i // 2, i % 2
        s = hg * HL * D
        wq_pk = np.ascontiguousarray(
            wq4[:, :, s:s + 512].reshape(CO, P, NPAIR, P)
            .transpose(2, 1, 0, 3))
        wk_pk = np.ascontiguousarray(
            wq4[:, :, C + s:C + s + 512].reshape(CO, P, NPAIR, P)
            .transpose(2, 1, 0, 3))
        wv_pk = np.ascontiguousarray(
            wq4[:, :, 2 * C + s:2 * C + s + 512].transpose(1, 0, 2))
        wo_pk = np.ascontiguousarray(
            wout_h[s:s + 512, :].reshape(EO, P, C).transpose(1, 0, 2))
        in_maps.append({"xbT": xT_h[b], "maskb": mask_f[b],
                        "wq_pk": wq_pk, "wk_pk": wk_pk, "wv_pk": wv_pk,
                        "wo_pk": wo_pk})
    return in_maps


def run_kernel(x, mask, w_qkv, w_out, b_out, trace=False):
    """Run on 8 cores; returns (full output [B,N,C], BassKernelResults)."""
    nc = _get_nc()
    in_maps = _make_in_maps(x, mask, w_qkv, w_out, b_out)
    res = run_bass_kernel_spmd(nc, in_maps, core_ids=list(range(N_CORES)),
                               trace=trace)
    bout_f = np.asarray(b_out).astype(np.float32)
    out = np.empty((B, N, C), dtype=np.float32)
    for b in range(B):
        acc = bout_f.copy()
        for i in (2 * b, 2 * b + 1):
            acc = acc + res.results[i]["y_a"].astype(np.float32) \
                      + res.results[i]["y_b"].astype(np.float32)
        out[b] = acc
    return out, res


def kernel(x, mask, w_qkv, w_out, b_out):
    os.environ.setdefault("BASS_NEVER_TRACE", "1")
    out, _ = run_kernel(x, mask, w_qkv, w_out, b_out, trace=False)
    return out


# revision 9
# speedup vs baseline: 1.4921x; 1.0714x over previous
"""Multi-head attention (B=4, N=1024, C=1024, H=16, D=64) on 8 Trainium2 cores.

Sharding: tensor-parallel over heads (the spec hint). Core i handles batch
b = i//2 and head group hg = i%2 (heads 8*hg..8*hg+7): it projects q/k/v for
its 8 heads over all 1024 rows, runs attention, and computes PARTIAL output
projections over its 512 e-dims (w_out rows 512*hg..+512), split into two
eo-halves (y_a: pairs 0-1, y_b: pairs 2-3) so most of the output projection
runs before the last softmax finishes. The host sums the four partials per
batch (the tensor-parallel all-reduce) and adds b_out. 6.45 GFLOP/core.

The kernel is explicitly software-pipelined at mo (key-tile) granularity.
The scalar engine's exp chain (32 x [128,2048] ACTIVATEs ~= 76us) and the
single 4-bank score psum force strict S(mo) -> exp(mo) -> S(mo+1)
alternation; every other PE unit (projections, AV, normalize-broadcast,
output partials) is interleaved between S groups so the PE never idles and
the HAM clock never re-throttles. Dummy warmup matmuls run during the input
DMA so the first real matmul is already at 2.4 GHz. Non-urgent input DMAs go
through gpsimd SWDGE to keep the sync HWDGE queue short for the
critical-path loads.
"""

import os

import numpy as np

import concourse.bacc as bacc
import concourse.mybir as mybir
import concourse.tile as tile
from concourse.bass_utils import run_bass_kernel_spmd

F32 = mybir.dt.float32
F16 = mybir.dt.float16

B, N, C = 4, 1024, 1024
H, D = 16, 64
HL = 8            # heads per core
P = 128
MO = N // P       # 8 key m-tiles
CO = C // P       # 8 contraction tiles
EO = HL * D // P  # 4 e-tiles (local)
NB = N // P       # 8 output row tiles
NQ2 = 512         # psum free-dim tile
NPAIR = HL // 2   # 4 local head pairs
ATT_SCALE = D ** -0.5
N_CORES = 8


def build_nc():
    nc = bacc.Bacc()
    xbT = nc.declare_dram_parameter("xbT", [C, N], F16, isOutput=False)
    maskb = nc.declare_dram_parameter("maskb", [N], F32, isOutput=False)
    wq_pk = nc.declare_dram_parameter("wq_pk", [NPAIR, P, CO, P], F16,
                                      isOutput=False)
    wk_pk = nc.declare_dram_parameter("wk_pk", [NPAIR, P, CO, P], F16,
                                      isOutput=False)
    wv_pk = nc.declare_dram_parameter("wv_pk", [P, CO, HL * D], F16,
                                      isOutput=False)
    wo_pk = nc.declare_dram_parameter("wo_pk", [P, EO, C], F16, isOutput=False)
    y_a = nc.declare_dram_parameter("y_a", [N, C], F16, isOutput=True)
    y_b = nc.declare_dram_parameter("y_b", [N, C], F16, isOutput=True)

    xbT_t = xbT.rearrange("(co p) m -> p co m", p=P)
    ya_t = y_a.rearrange("(nb p) c -> p nb c", p=P)
    yb_t = y_b.rearrange("(nb p) c -> p nb c", p=P)

    with tile.TileContext(nc) as tc:
        with tc.tile_pool(name="consts", bufs=1) as consts, \
             tc.tile_pool(name="persist", bufs=1) as persist:
            ones_q = consts.tile([33, D], F16)       # K=1 broadcast lhsT
            nc.vector.memset(ones_q[:], 1.0)
            ones_row = consts.tile([1, NQ2], F16)    # warmup rhs
            nc.vector.memset(ones_row[:], 1.0)
            onesH = consts.tile([P, HL], F16)
            nc.vector.memset(onesH[:], 1.0)
            mask_sb = consts.tile([P, MO], F32)

            qT = persist.tile([P, NPAIR, N], F16)
            kT = persist.tile([P, NPAIR, N], F16)
            v_sb = persist.tile([P, MO, HL, D + 1], F16)
            o_sb = persist.tile([P, EO, N], F16)
            wo = persist.tile([P, EO, C], F16)

            with tc.tile_pool(name="xT_pool", bufs=1) as xT_pool, \
                 tc.tile_pool(name="w_pool", bufs=4) as w_pool, \
                 tc.tile_pool(name="wv_pool", bufs=1) as wv_pool, \
                 tc.tile_pool(name="E_pool", bufs=3) as E_pool, \
                 tc.tile_pool(name="den_pool", bufs=3) as den_pool, \
                 tc.tile_pool(name="rcp_pool", bufs=2) as rcp_pool, \
                 tc.tile_pool(name="ou_pool", bufs=4) as ou_pool, \
                 tc.tile_pool(name="ys_pool", bufs=3) as ys_pool, \
                 tc.tile_pool(name="ps_proj", bufs=2, space="PSUM") as ps_proj, \
                 tc.tile_pool(name="ps_s", bufs=1, space="PSUM") as ps_s, \
                 tc.tile_pool(name="ps_av", bufs=2, space="PSUM") as ps_av:
                xT = xT_pool.tile([P, CO, N], F16)

                # ---- HAM warmup: PE busy during input DMA ----
                for i in range(8):
                    pw = ps_av.tile([64, NQ2], F32, tag="av", name=f"warm{i}")
                    nc.tensor.matmul(pw[:], ones_q[0:1, :], ones_row[:],
                                     start=True, stop=True)

                # ---- critical-path DMAs on sync HWDGE ----
                wk0 = w_pool.tile([P, CO, P], F16, tag="wqk", name="wk0")
                nc.sync.dma_start(wk0[:], wk_pk[0])
                nc.sync.dma_start(xT[:, 0, :], xbT_t[:, 0, :])
                wq0 = w_pool.tile([P, CO, P], F16, tag="wqk", name="wq0")
                nc.sync.dma_start(wq0[:], wq_pk[0])
                for co in range(1, CO):
                    nc.sync.dma_start(xT[:, co, :], xbT_t[:, co, :])
                # ---- bulk prefetch on the scalar HWDGE queue ----
                nc.scalar.dma_start(mask_sb[:],
                                    maskb.rearrange("(o p) -> p o", p=P))
                wv = wv_pool.tile([P, CO, HL * D], F16)
                nc.scalar.dma_start(wv[:], wv_pk[:])

                wtiles = {0: (wk0, wq0)}

                def prefetch_w(t):
                    wk = w_pool.tile([P, CO, P], F16, tag="wqk",
                                     name=f"wk{t}")
                    nc.sync.dma_start(wk[:], wk_pk[t])
                    wq = w_pool.tile([P, CO, P], F16, tag="wqk",
                                     name=f"wq{t}")
                    nc.sync.dma_start(wq[:], wq_pk[t])
                    wtiles[t] = (wk, wq)

                def emit_proj(t, kind, nh):
                    # one [128, 512] psum group of the k/q projection
                    w = wtiles[t][0 if kind == "k" else 1]
                    dst = kT if kind == "k" else qT
                    pp = ps_proj.tile([P, NQ2], F32, tag="pp",
                                      name=f"p{kind}{t}_{nh}")
                    for co in range(CO):
                        nc.tensor.matmul(
                            pp[:], w[:, co, :],
                            xT[:, co, nh * NQ2:(nh + 1) * NQ2],
                            start=(co == 0), stop=(co == CO - 1))
                    nc.vector.tensor_copy(
                        dst[:, t, nh * NQ2:(nh + 1) * NQ2], pp[:])

                def emit_v(mo):
                    pv = ps_proj.tile([P, NQ2], F32, tag="pp",
                                      name=f"pv{mo}")
                    for co in range(CO):
                        nc.tensor.matmul(
                            pv[:], xT[:, co, mo * P:(mo + 1) * P],
                            wv[:, co, :],
                            start=(co == 0), stop=(co == CO - 1))
                    nc.vector.tensor_scalar_mul(
                        v_sb[:, mo, :, 0:D],
                        pv[:].rearrange("p (h d) -> p h d", d=D),
                        mask_sb[:, mo:mo + 1])
                    nc.vector.tensor_scalar_mul(
                        v_sb[:, mo, :, D], onesH[:], mask_sb[:, mo:mo + 1])

                def emit_s(t, mo, E_pair):
                    pss = ps_s.tile([P, 4 * NQ2], F32, tag="ps_s",
                                    name=f"pss{t}_{mo}")
                    for nh in range(2):
                        for j in range(2):
                            pb = 64 * j
                            nc.tensor.matmul(
                                pss[:, (2 * j + nh) * NQ2:
                                    (2 * j + nh + 1) * NQ2],
                                kT[pb:pb + 64, t, mo * P:(mo + 1) * P],
                                qT[pb:pb + 64, t, nh * NQ2:(nh + 1) * NQ2],
                                start=True, stop=True)
                    nc.scalar.activation(
                        E_pair[:, mo, :, :], pss[:],
                        mybir.ActivationFunctionType.Exp, scale=ATT_SCALE)

                norm_state = {}

                def emit_avc(t, j, nh, E_pair):
                    # one AV accumulation chain (one head, one n-half)
                    if t not in norm_state:
                        o_un = ou_pool.tile([P, N], F16, tag="ou",
                                            name=f"ou{t}")
                        den = den_pool.tile([33, N], F32, tag="den",
                                            name=f"den{t}")
                        norm_state[t] = (o_un, den)
                    o_un, den = norm_state[t]
                    h = 2 * t + j
                    pb = 64 * j
                    pav = ps_av.tile([P, NQ2], F32, tag="av",
                                     name=f"pav{h}_{nh}")
                    for mo in range(MO):
                        nc.tensor.matmul(
                            pav[0:D + 1, :], v_sb[:, mo, h, :],
                            E_pair[:, mo, j, nh * NQ2:(nh + 1) * NQ2],
                            start=(mo == 0), stop=(mo == MO - 1))
                    nc.vector.tensor_copy(
                        o_un[pb:pb + 64, nh * NQ2:(nh + 1) * NQ2],
                        pav[0:D, :])
                    nc.vector.tensor_copy(
                        den[32 * j:32 * j + 1, nh * NQ2:(nh + 1) * NQ2],
                        pav[D:D + 1, :])

                def emit_norm_rcp(t):
                    o_un, den = norm_state[t]
                    rcp32 = rcp_pool.tile([33, N], F32, tag="rcp32",
                                          name=f"rcp32_{t}")
                    nc.vector.reciprocal_approx_fast(rcp32[:], den[:])
                    rcp16 = rcp_pool.tile([33, N], F16, tag="rcp16",
                                          name=f"rcp16_{t}")
                    nc.vector.tensor_copy(rcp16[:], rcp32[:])
                    norm_state[t] = (o_un, den, rcp16)

                def emit_norm_bc(t):
                    o_un, den, rcp16 = norm_state[t]
                    for nh in range(2):
                        for j in range(2):
                            pb = 64 * j
                            pbc = ps_proj.tile([64, NQ2], F32, tag="pp",
                                               name=f"pbc{t}_{j}_{nh}")
                            nc.tensor.matmul(
                                pbc[:], ones_q[32 * j:32 * j + 1, :],
                                rcp16[32 * j:32 * j + 1,
                                      nh * NQ2:(nh + 1) * NQ2],
                                start=True, stop=True,
                                tile_position=(32 * j, 0))
                            nc.vector.tensor_mul(
                                o_sb[pb:pb + 64, t, nh * NQ2:(nh + 1) * NQ2],
                                pbc[:],
                                o_un[pb:pb + 64, nh * NQ2:(nh + 1) * NQ2])

                def emit_norm(t):
                    emit_norm_rcp(t)
                    emit_norm_bc(t)

                def emit_out(part, nb, evac="v"):
                    # both ch halves of one output row block + one DMA
                    yt = ya_t if part == 0 else yb_t
                    ysb = ys_pool.tile([P, N], F16, tag="ysb",
                                       name=f"ysb{part}_{nb}")
                    for ch in range(2):
                        py = ps_proj.tile([P, NQ2], F32, tag="pp",
                                          name=f"py{part}_{nb}_{ch}")
                        for ei in range(2):
                            eo = 2 * part + ei
                            nc.tensor.matmul(
                                py[:], o_sb[:, eo, nb * P:(nb + 1) * P],
                                wo[:, eo, ch * NQ2:(ch + 1) * NQ2],
                                start=(ei == 0), stop=(ei == 1))
                        dst = ysb[:, ch * NQ2:(ch + 1) * NQ2]
                        if evac == "s" and ch == 0:
                            nc.scalar.copy(dst, py[:])
                        else:
                            nc.vector.tensor_copy(dst, py[:])
                    eng = nc.sync if (part == 0 or nb % 2 == 0) else nc.scalar
                    eng.dma_start(yt[:, nb, :], ysb[:])

                def fetch_wo():
                    for eo in range(EO):
                        nc.sync.dma_start(wo[:, eo, :], wo_pk[:, eo, :])

                # ---- prologue: minimal work before the first exp ----
                emit_proj(0, "k", 0)
                emit_proj(0, "q", 0)
                emit_proj(0, "q", 1)

                Es = {t: E_pool.tile([P, MO, 2, N], F16, tag="E",
                                     name=f"E{t}")
                      for t in range(NPAIR)}

                # fillers emitted after each S(t, mo) group; deadlines:
                # kq(t) before S(t,0); k_nh1(0) before S(0,4); v before AV0;
                # AV(t) before exp(t+3) (E bufs=3); norm(t) before out parts.
                F = {
                    (0, 0): [lambda: emit_proj(0, "k", 1)],
                    (0, 1): [lambda: prefetch_w(1), lambda: emit_v(0)],
                    (0, 2): [lambda: emit_v(1)],
                    (0, 3): [lambda: emit_v(2)],
                    (0, 4): [lambda: emit_proj(1, "k", 0)],
                    (0, 5): [lambda: emit_proj(1, "k", 1)],
                    (0, 6): [lambda: emit_proj(1, "q", 0)],
                    (0, 7): [lambda: emit_proj(1, "q", 1)],
                    (1, 0): [fetch_wo, lambda: emit_v(3)],
                    (1, 1): [lambda: emit_v(4)],
                    (1, 2): [lambda: emit_v(5)],
                    (1, 3): [lambda: prefetch_w(2), lambda: emit_v(6)],
                    (1, 4): [lambda: emit_proj(2, "k", 0)],
                    (1, 5): [lambda: emit_proj(2, "k", 1)],
                    (1, 6): [lambda: emit_proj(2, "q", 0)],
                    (1, 7): [lambda: emit_proj(2, "q", 1)],
                    (2, 0): [lambda: emit_v(7),
                             lambda: emit_avc(0, 0, 0, Es[0])],
                    (2, 1): [lambda: emit_avc(0, 0, 1, Es[0]),
                             lambda: emit_avc(0, 1, 0, Es[0])],
                    (2, 2): [lambda: emit_avc(0, 1, 1, Es[0]),
                             lambda: emit_norm_rcp(0)],
                    (2, 3): [lambda: prefetch_w(3),
                             lambda: emit_norm_bc(0),
                             lambda: emit_avc(1, 0, 0, Es[1])],
                    (2, 4): [lambda: emit_proj(3, "k", 0),
                             lambda: emit_avc(1, 0, 1, Es[1])],
                    (2, 5): [lambda: emit_proj(3, "k", 1),
                             lambda: emit_avc(1, 1, 0, Es[1])],
                    (2, 6): [lambda: emit_proj(3, "q", 0),
                             lambda: emit_avc(1, 1, 1, Es[1]),
                             lambda: emit_norm_rcp(1)],
                    (2, 7): [lambda: emit_proj(3, "q", 1),
                             lambda: emit_norm_bc(1)],
                    (3, 0): [lambda: emit_out(0, 0)],
                    (3, 1): [lambda: emit_out(0, 1)],
                    (3, 2): [lambda: emit_out(0, 2)],
                    (3, 3): [lambda: emit_out(0, 3)],
                    (3, 4): [lambda: emit_avc(2, 0, 0, Es[2]),
                             lambda: emit_avc(2, 0, 1, Es[2]),
                             lambda: emit_out(0, 4)],
                    (3, 5): [lambda: emit_avc(2, 1, 0, Es[2]),
                             lambda: emit_avc(2, 1, 1, Es[2]),
                             lambda: emit_norm_rcp(2)],
                    (3, 6): [lambda: emit_norm_bc(2),
                             lambda: emit_out(0, 5)],
                    (3, 7): [lambda: emit_avc(3, 0, 0, Es[3]),
                             lambda: emit_avc(3, 0, 1, Es[3]),
                             lambda: emit_avc(3, 1, 0, Es[3]),
                             lambda: emit_avc(3, 1, 1, Es[3]),
                             lambda: emit_norm_rcp(3)],
                }

                for t in range(NPAIR):
                    for mo in range(MO):
                        emit_s(t, mo, Es[t])
                        for f in F.get((t, mo), []):
                            f()

                # ---- tail ----
                emit_out(0, 6)
                emit_out(0, 7)
                emit_norm_bc(3)
                for nb in range(NB):
                    emit_out(1, nb, evac="s")

    nc.finalize()
    return nc


_NC_CACHE = None


def _get_nc():
    global _NC_CACHE
    if _NC_CACHE is None:
        _NC_CACHE = build_nc()
    return _NC_CACHE


def _make_in_maps(x, mask, w_qkv, w_out, b_out):
    x = np.ascontiguousarray(np.asarray(x), dtype=np.float32)
    mask_f = np.asarray(mask).astype(np.float32)
    wqkv_h = np.asarray(w_qkv).astype(np.float16)
    wout_h = np.asarray(w_out).astype(np.float16)
    # per head group hg: q cols hg*512..+512 of wqkv[:, 0:C], k of [C:2C],
    # v of [2C:3C]; w_out rows hg*512..+512. Packed so every DMA is
    # contiguous per partition: C = (co p).
    wq4 = wqkv_h.reshape(CO, P, 3 * H * D)
    xT_h = [np.ascontiguousarray(x[b].T.astype(np.float16)) for b in range(B)]
    in_maps = []
    for i in range(N_CORES):
        b, hg = i // 2, i % 2
        s = hg * HL * D
        wq_pk = np.ascontiguousarray(
            wq4[:, :, s:s + 512].reshape(CO, P, NPAIR, P)
            .transpose(2, 1, 0, 3))
        wk_pk = np.ascontiguousarray(
            wq4[:, :, C + s:C + s + 512].reshape(CO, P, NPAIR, P)
            .transpose(2, 1, 0, 3))
        wv_pk = np.ascontiguousarray(
            wq4[:, :, 2 * C + s:2 * C + s + 512].transpose(1, 0, 2))
        wo_pk = np.ascontiguousarray(
            wout_h[s:s + 512, :].reshape(EO, P, C).transpose(1, 0, 2))
        in_maps.append({"xbT": xT_h[b], "maskb": mask_f[b],
                        "wq_pk": wq_pk, "wk_pk": wk_pk, "wv_pk": wv_pk,
                        "wo_pk": wo_pk})
    return in_maps


def run_kernel(x, mask, w_qkv, w_out, b_out, trace=False):
    """Run on 8 cores; returns (full output [B,N,C], BassKernelResults)."""
    nc = _get_nc()
    in_maps = _make_in_maps(x, mask, w_qkv, w_out, b_out)
    res = run_bass_kernel_spmd(nc, in_maps, core_ids=list(range(N_CORES)),
                               trace=trace)
    bout_f = np.asarray(b_out).astype(np.float32)
    out = np.empty((B, N, C), dtype=np.float32)
    for b in range(B):
        acc = bout_f.copy()
        for i in (2 * b, 2 * b + 1):
            acc = acc + res.results[i]["y_a"].astype(np.float32) \
                      + res.results[i]["y_b"].astype(np.float32)
        out[b] = acc
    return out, res


def kernel(x, mask, w_qkv, w_out, b_out):
    os.environ.setdefault("BASS_NEVER_TRACE", "1")
    out, _ = run_kernel(x, mask, w_qkv, w_out, b_out, trace=False)
    return out
